# revision 34
# baseline (speedup 1.0000x reference)
"""Multi-head causal self-attention (B=2, T=2048, D=2048, 16 heads, RoPE)
on 8 Trainium2 NeuronCores — v4.

Changes vs v3 (470us):
* Lag schedule: head-1 attention units trail head-0 by one chunk inside
  both batch loops, so every chunk iteration carries QKV + ~2 attention
  units and the post-loop tail is a single unit.
* Four quarter-size AllToAlls, one per (head-parity, batch), each over
  [8*128, 256]-token blocks.  They fire as soon as their 4 units finish
  (~150/190/288/300us) and pipeline on the CC engine, so the tail
  collective is 512KB instead of 1MB and starts far earlier.
* Out-projection runs as four quarter-passes (one per round): parity-0
  parks partial sums in fp16, parity-1 adds and stores.  Each core now
  owns one 256-token slice of BOTH batches (y rows = [b0 slice; b1
  slice]); the host gather interleaves accordingly.
* tc.no_sync_barrier() before each quarter-pass emission: the Tile
  scheduler under-models collective latency and otherwise hoists
  pass matmuls into the attention PE stream, where their LDWEIGHTS
  stall the strict-FIFO PE queue (v3 lost 28us to this).
"""

import numpy as np

B = 2
T = 2048
D = 2048
H = 16
HD = 128
NCORES = 8
HPC = H // NCORES   # heads per core (2)
NKT = D // 128      # contraction tiles (16)
TCH = 512           # t-chunk width
NCH = T // TCH      # chunks per batch (4)
W = HPC * HD        # per-core qkv feature width (256)
QT = T // NCORES    # per-core token slice for out-proj (256)
RPC = B * QT        # output rows per core (512)
SCALE = 1.0 / np.sqrt(HD)
EXPB = -float(np.log(16.0))   # exp pre-bias keeps the fp16 denom small

_CACHE = {}


def _build_module():
    import concourse.bacc as bacc
    import concourse.mybir as mybir
    import concourse.tile as tile

    F32 = mybir.dt.float32
    BF16 = mybir.dt.bfloat16
    FP16 = mybir.dt.float16
    ADD = mybir.AluOpType.add
    MULT = mybir.AluOpType.mult
    AF = mybir.ActivationFunctionType

    nc = bacc.Bacc("TRN2", target_bir_lowering=False, debug=False,
                   num_devices=NCORES)

    # ---- I/O ----
    xT = nc.dram_tensor("xT", [B, D, T], BF16, kind="ExternalInput")
    wq = nc.dram_tensor("wq", [D, W], BF16, kind="ExternalInput")
    wk = nc.dram_tensor("wk", [D, W], BF16, kind="ExternalInput")
    wv = nc.dram_tensor("wv", [D, W], BF16, kind="ExternalInput")
    bqk = nc.dram_tensor("bqk", [2 * HPC, HD, 1], F32, kind="ExternalInput")
    wo = nc.dram_tensor("wo", [D, D], BF16, kind="ExternalInput")
    bo = nc.dram_tensor("bo", [1, D], FP16, kind="ExternalInput")
    cosT = nc.dram_tensor("cosT", [HD, T], FP16, kind="ExternalInput")
    sinT = nc.dram_tensor("sinT", [HD, T], FP16, kind="ExternalInput")
    pt = nc.dram_tensor("pt", [HD, HD], FP16, kind="ExternalInput")
    maskT = nc.dram_tensor("maskT", [HD, HD], BF16, kind="ExternalInput")
    onec16 = nc.dram_tensor("onec16", [HD, 1], FP16, kind="ExternalInput")
    oner = nc.dram_tensor("oner", [1, HD], FP16, kind="ExternalInput")
    expb = nc.dram_tensor("expb", [HD, 1], F32, kind="ExternalInput")
    y = nc.dram_tensor("y", [RPC, D], FP16, kind="ExternalOutput")

    with tile.TileContext(nc) as tc:
        frees = []

        def single(shape, dtype, name, flist=frees):
            t, free = tc.tile(shape, dtype, name=name)
            flist.append(free)
            return t

        pt_sb = single([HD, HD], FP16, "pt_sb")
        mask_sb = single([HD, HD], BF16, "mask_sb")
        onec_sb = single([HD, 1], FP16, "onec_sb")
        oner_sb = single([1, HD], FP16, "oner_sb")
        bqk_sb = single([HD, 2 * HPC], F32, "bqk_sb")
        expb_sb = single([HD, 1], F32, "expb_sb")
        bo_sb = single([1, D], FP16, "bo_sb")

        with tc.tile_pool(name="dram", bufs=1, space="DRAM") as dram:
            # round (hl, b): block p = head(2s+hl) of batch b, token slice
            # [p*QT:(p+1)*QT]
            bounce_in = [[dram.tile([NCORES * HD, QT], BF16,
                                    name=f"bi{hl}{b}") for b in range(B)]
                         for hl in range(HPC)]
            bounce_out = [[dram.tile([NCORES * HD, QT], BF16,
                                     name=f"bu{hl}{b}") for b in range(B)]
                          for hl in range(HPC)]
            warm_in = dram.tile([NCORES, 128], BF16, name="warm_in")
            warm_out = dram.tile([NCORES, 128], BF16, name="warm_out")

            with tc.tile_pool(name="qk_ps", bufs=3, space="PSUM") as qk_ps, \
                 tc.tile_pool(name="rot_ps", bufs=2, space="PSUM") as rot_ps, \
                 tc.tile_pool(name="st_ps", bufs=2, space="PSUM") as st_ps, \
                 tc.tile_pool(name="ot_ps", bufs=1, space="PSUM") as ot_ps, \
                 tc.tile_pool(name="et", bufs=24) as et_pool, \
                 tc.tile_pool(name="ets", bufs=2) as ets_pool, \
                 tc.tile_pool(name="nrm", bufs=4) as nrm_pool, \
                 tc.tile_pool(name="oto", bufs=2) as oto_pool, \
                 tc.tile_pool(name="wop", bufs=32) as wo_pool, \
                 tc.tile_pool(name="ocp", bufs=16) as oc_pool, \
                 tc.tile_pool(name="osp", bufs=16) as osp_pool, \
                 tc.tile_pool(name="ozp", bufs=2) as oz_pool:

                # stack-ordered singles: tiles alive into the post-loop
                # tail at the bottom, so the rest frees before wo1 opens.
                v_st = [[None] * HPC for _ in range(B)]
                q_st = [[None] * HPC for _ in range(B)]
                k_st = [[None] * HPC for _ in range(B)]
                keepfrees = []
                v_st[1][1] = single([128, T], BF16, "v11", keepfrees)
                q_st[1][1] = single([128, T], FP16, "q11", keepfrees)
                k_st[1][1] = single([128, T], FP16, "k11", keepfrees)
                deadfrees = []
                v_st[0][0] = single([128, T], BF16, "v00", deadfrees)
                v_st[0][1] = single([128, T], BF16, "v01", deadfrees)
                v_st[1][0] = single([128, T], BF16, "v10", deadfrees)
                q_st[0][0] = single([128, T], FP16, "q00", deadfrees)
                q_st[0][1] = single([128, T], FP16, "q01", deadfrees)
                q_st[1][0] = single([128, T], FP16, "q10", deadfrees)
                k_st[0][0] = single([128, T], FP16, "k00", deadfrees)
                k_st[0][1] = single([128, T], FP16, "k01", deadfrees)
                k_st[1][0] = single([128, T], FP16, "k10", deadfrees)
                wfrees = []
                wq_sb = single([128, NKT * W], BF16, "wq_sb", wfrees)
                wk_sb = single([128, NKT * W], BF16, "wk_sb", wfrees)
                wv_sb = single([128, NKT * W], BF16, "wv_sb", wfrees)

                # out-proj weight tiles: wts[(s, fc, hl)] = wo rows of head
                # 2s+hl, out-cols [fc*TCH:(fc+1)*TCH]
                wts = {}

                def wo_load(s, fc, hl, eng, pool):
                    kt = HPC * s + hl
                    t_ = pool.tile([128, TCH], BF16,
                                   name=f"wo{kt}_{fc}", tag="wo")
                    eng.dma_start(
                        t_[:],
                        wo.ap()[kt * 128:(kt + 1) * 128,
                                fc * TCH:(fc + 1) * TCH])
                    wts[s, fc, hl] = t_

                # received attention outputs: oc[(hl, b)][s]
                oc = {}

                def oc_load(hl, b):
                    tl = []
                    for s in range(NCORES):
                        t_ = oc_pool.tile([128, QT], BF16,
                                          name=f"oc{hl}{b}_{s}", tag="oc")
                        nc.gpsimd.dma_start(
                            t_[:],
                            bounce_out[hl][b][s * HD:(s + 1) * HD, :])
                        tl.append(t_)
                    oc[hl, b] = tl

                # ---------- emission helpers ----------
                st_state = {}

                def gen_A(b, hl, c):
                    """Scores + exp + fp16 running denominator (one unit).
                    Yields once per k-tile; tail emits den/recip/rcr."""
                    nk = 4 * c + 4
                    q0 = c * TCH
                    ets = ets_pool.tile([128, TCH], FP16,
                                        name=f"ets{b}{hl}{c}", tag="ets")
                    et_list = []
                    for k in range(nk):
                        off = max(0, (k - 4 * c) * 128)
                        st = st_ps.tile([128, TCH], F32,
                                        name=f"st{b}{hl}{c}{k}", tag="st")
                        nc.tensor.matmul(
                            st[:, off:TCH],
                            k_st[b][hl][:, k * 128:(k + 1) * 128],
                            q_st[b][hl][:, q0 + off:q0 + TCH],
                            start=True, stop=True,
                            skip_group_check=True)
                        et = et_pool.tile([128, TCH], BF16,
                                          name=f"et{b}{hl}{c}{k}", tag="et")
                        nc.scalar.activation(
                            et[:, off:TCH], st[:, off:TCH],
                            AF.Exp, bias=expb_sb[:, 0:1],
                            scale=float(SCALE))
                        if k >= 4 * c:
                            nc.vector.tensor_tensor(
                                et[:, off:off + 128],
                                et[:, off:off + 128], mask_sb[:], MULT)
                        if k == 0:
                            nc.vector.tensor_copy(ets[:], et[:])
                        else:
                            nc.vector.tensor_tensor(
                                ets[:, off:TCH], ets[:, off:TCH],
                                et[:, off:TCH], ADD)
                        et_list.append((et, off))
                        yield
                    den = rot_ps.tile([1, TCH], F32,
                                      name=f"den{b}{hl}{c}", tag="rot")
                    nc.tensor.matmul(den[0:1, :], onec_sb[:], ets[:],
                                     start=True, stop=True,
                                     skip_group_check=True)
                    rc = nrm_pool.tile([1, TCH], F32,
                                       name=f"rc{b}{hl}{c}", tag="rc")
                    rscr = nrm_pool.tile([1, TCH], F32,
                                         name=f"rs{b}{hl}{c}", tag="rc")
                    nc.vector.reciprocal_approx_accurate(
                        rc[:], den[0:1, :], rscr[:])
                    rcr = nrm_pool.tile([1, TCH], FP16,
                                        name=f"rr{b}{hl}{c}", tag="rcr")
                    nc.scalar.copy(rcr[:], rc[:])
                    st_state["unit"] = (b, hl, c, et_list, rcr)

                def gen_B(unit):
                    """AV accumulation + normalization + A2A scatter."""
                    b, hl, c, et_list, rcr = unit
                    nk = len(et_list)
                    bc = rot_ps.tile([128, TCH], F32,
                                     name=f"bc{b}{hl}{c}", tag="rot")
                    nc.tensor.matmul(bc[:], oner_sb[:], rcr[:],
                                     start=True, stop=True,
                                     skip_group_check=True)
                    bcs = nrm_pool.tile([128, TCH], F32,
                                        name=f"bs{b}{hl}{c}", tag="bcs")
                    nc.scalar.copy(bcs[:], bc[:])
                    ot = ot_ps.tile([128, TCH], F32,
                                    name=f"ot{b}{hl}{c}", tag="ot")
                    for k in range(nk):
                        et, off = et_list[k]
                        nc.tensor.matmul(
                            ot[:, off:TCH],
                            v_st[b][hl][:, k * 128:(k + 1) * 128],
                            et[:, off:TCH],
                            start=(k == 0), stop=(k == nk - 1),
                            skip_group_check=True)
                        yield
                    otn = oto_pool.tile([128, TCH], BF16,
                                        name=f"on{b}{hl}{c}", tag="otn")
                    nc.vector.tensor_tensor(otn[:], ot[:], bcs[:], MULT)
                    # chunk c covers token blocks 2c, 2c+1 of this batch
                    nc.sync.dma_start(
                        bounce_in[hl][b][2 * c * HD:(2 * c + 1) * HD, :],
                        otn[:, 0:QT])
                    nc.sync.dma_start(
                        bounce_in[hl][b][(2 * c + 1) * HD:(2 * c + 2) * HD, :],
                        otn[:, QT:TCH])

                def drive(a_gen, b_gen, c_gen=None):
                    """Alternate B, A (and optional filler) steps — B
                    first: its deps are older, so the PE queue head never
                    blocks long."""
                    gens = [g for g in (b_gen, a_gen, c_gen)
                            if g is not None]
                    while gens:
                        for g in list(gens):
                            try:
                                next(g)
                            except StopIteration:
                                gens.remove(g)

                def fire_round(hl, b):
                    nc.gpsimd.collective_compute(
                        "AllToAll",
                        mybir.AluOpType.bypass,
                        replica_groups=[list(range(NCORES))],
                        ins=[bounce_in[hl][b][:].opt()],
                        outs=[bounce_out[hl][b][:].opt()],
                    )

                pending = [None]

                def run_unit(b, hl, c):
                    a = gen_A(b, hl, c)
                    bgen = (gen_B(pending[0])
                            if pending[0] is not None else None)
                    drive(a, bgen)
                    pending[0] = st_state["unit"]

                def quarter_pass_gen(hl, b, fcs=None):
                    first = (hl == 0)
                    for fc in (fcs if fcs is not None else range(D // TCH)):
                        for rt in range(QT // 128):
                            po = qk_ps.tile([128, TCH], F32,
                                            name=f"po{hl}{b}{fc}{rt}",
                                            tag="qk")
                            if first:
                                nc.tensor.matmul(
                                    po[:], oner_sb[:],
                                    bo_sb[0:1, fc * TCH:(fc + 1) * TCH],
                                    start=True, stop=False,
                                    skip_group_check=True)
                            for s in range(NCORES):
                                nc.tensor.matmul(
                                    po[:],
                                    oc[hl, b][s][:, rt * 128:(rt + 1) * 128],
                                    wts[s, fc, hl][:],
                                    start=(not first and s == 0),
                                    stop=(s == NCORES - 1),
                                    skip_group_check=True)
                            if first:
                                osp_t = osp_pool.tile(
                                    [128, TCH], FP16,
                                    name=f"os{b}{fc}{rt}", tag="osp")
                                if (fc + rt) % 2:
                                    nc.scalar.copy(osp_t[:], po[:])
                                else:
                                    nc.vector.tensor_copy(osp_t[:], po[:])
                                osp[b, fc, rt] = osp_t
                            else:
                                oz = oz_pool.tile(
                                    [128, TCH], FP16,
                                    name=f"oz{b}{fc}{rt}", tag="oz")
                                nc.vector.tensor_tensor(
                                    oz[:], po[:], osp[b, fc, rt][:], ADD)
                                r0 = b * QT + rt * 128
                                nc.sync.dma_start(
                                    y.ap()[r0:r0 + 128,
                                           fc * TCH:(fc + 1) * TCH],
                                    oz[:])
                            yield

                def quarter_pass(hl, b, fcs=None):
                    for _ in quarter_pass_gen(hl, b, fcs):
                        pass

                osp = {}

                with tc.tile_pool(name="xt", bufs=17) as xt_pool, \
                     tc.tile_pool(name="cs", bufs=4) as cs_pool, \
                     tc.tile_pool(name="tmp", bufs=4) as tmp_pool:

                    cs_cache = {}

                    def p1_chunk(b, j):
                        """QKV projection + RoPE for chunk j of batch b."""
                        tr = slice(j * TCH, (j + 1) * TCH)
                        if b == 0 and j == 0:
                            nc.sync.dma_start(pt_sb[:], pt.ap()[:, :])
                            for i in range(2 * HPC):
                                nc.gpsimd.dma_start(bqk_sb[:, i:i + 1],
                                                    bqk.ap()[i])
                            nc.gpsimd.dma_start(oner_sb[:], oner.ap()[:, :])
                        xt = []
                        for kt in range(NKT):
                            xtile = xt_pool.tile([128, TCH], BF16,
                                                 name=f"xt{b}{j}_{kt}",
                                                 tag="xt")
                            xeng = (nc.gpsimd if (b == 0 and j == 0
                                                  and kt >= 8) else nc.sync)
                            xeng.dma_start(
                                xtile[:],
                                xT.ap()[b, kt * 128:(kt + 1) * 128, tr])
                            xt.append(xtile)
                            if b == 0 and j == 0 and kt == 0:
                                # issue order follows the consumption
                                # order of the first chains
                                def wload(wsb, wdr, k0, k1):
                                    if k1 - k0 == 1:
                                        nc.scalar.dma_start(
                                            wsb[:, k0 * W:k1 * W],
                                            wdr.ap()[k0 * 128:k1 * 128, :])
                                        return
                                    dst = wsb[:, k0 * W:k1 * W]
                                    dst = dst.rearrange(
                                        "p (k w) -> p k w", k=k1 - k0)
                                    srcw = wdr.ap()[k0 * 128:k1 * 128, :]
                                    srcw = srcw.rearrange(
                                        "(k p) w -> p k w", p=128)
                                    nc.scalar.dma_start(dst, srcw)
                                for wsb, wdr, k0, k1 in (
                                        (wq_sb, wq, 0, 1),
                                        (wq_sb, wq, 1, 6),
                                        (wk_sb, wk, 0, 1),
                                        (wq_sb, wq, 6, 11),
                                        (wq_sb, wq, 11, 16),
                                        (wk_sb, wk, 1, 6),
                                        (wk_sb, wk, 6, 11),
                                        (wk_sb, wk, 11, 16),
                                        (wv_sb, wv, 0, 6),
                                        (wv_sb, wv, 6, 11),
                                        (wv_sb, wv, 11, 16)):
                                    wload(wsb, wdr, k0, k1)
                        g = 4 * b + j
                        loads = ([0, 1] if g == 0 else
                                 [g + 1] if g + 1 < 2 * NCH else [])
                        for gl in loads:
                            jl = gl % NCH
                            trl = slice(jl * TCH, (jl + 1) * TCH)
                            cos_l = cs_pool.tile([HD, TCH], FP16,
                                                 name=f"cos{gl}", tag="cs")
                            nc.gpsimd.dma_start(cos_l[:], cosT.ap()[:, trl])
                            sin_l = cs_pool.tile([HD, TCH], FP16,
                                                 name=f"sin{gl}", tag="cs")
                            nc.gpsimd.dma_start(sin_l[:], sinT.ap()[:, trl])
                            cs_cache[gl] = (cos_l, sin_l)
                        cos_c, sin_c = cs_cache.pop(g)
                        if b == 0 and j == 0:
                            nc.gpsimd.dma_start(mask_sb[:], maskT.ap()[:, :])
                            nc.gpsimd.dma_start(onec_sb[:], onec16.ap()[:, :])
                            nc.gpsimd.dma_start(expb_sb[:], expb.ap()[:, :])
                            nc.scalar.dma_start(bo_sb[:], bo.ap()[:, :])
                        elif 1 <= g <= 5:
                            # prefetch parity-0 out-proj weights, spread
                            # thin so collectives never compete with a
                            # concentrated DMA burst
                            todo = [(s, fc) for fc in range(D // TCH)
                                    for s in range(NCORES)][(g - 1) * 7:
                                                            g * 7]
                            for s, fc in todo:
                                wo_load(s, fc, 0, nc.gpsimd, wo_pool)

                        for which, w_sb, store in (
                                ("q", wq_sb, q_st[b]), ("k", wk_sb, k_st[b])):
                            for hl in range(HPC):
                                ps = qk_ps.tile([128, TCH], F32,
                                                name=f"{which}p{b}{j}{hl}",
                                                tag="qk")
                                for kt in range(NKT):
                                    col = kt * W + hl * HD
                                    nc.tensor.matmul(
                                        ps[:], w_sb[:, col:col + HD],
                                        xt[kt][:],
                                        start=(kt == 0),
                                        stop=(kt == NKT - 1),
                                        skip_group_check=True)
                                bcol = hl if which == "q" else HPC + hl
                                qtmp = tmp_pool.tile(
                                    [128, TCH], FP16,
                                    name=f"{which}t{b}{j}{hl}", tag="tmp")
                                nc.scalar.activation(
                                    qtmp[:], ps[:], AF.Identity,
                                    bias=bqk_sb[:, bcol:bcol + 1], scale=1.0)
                                rp = rot_ps.tile([128, TCH], F32,
                                                 name=f"rp{b}{j}{hl}{which}",
                                                 tag="rot")
                                nc.tensor.matmul(rp[:], pt_sb[:], qtmp[:],
                                                 start=True, stop=True,
                                                 skip_group_check=True)
                                t1 = tmp_pool.tile([128, TCH], FP16,
                                                   name=f"t1_{b}{j}{hl}",
                                                   tag="tmp")
                                nc.vector.tensor_tensor(
                                    t1[:], qtmp[:], cos_c[:], MULT)
                                t2 = tmp_pool.tile([128, TCH], FP16,
                                                   name=f"t2_{b}{j}{hl}",
                                                   tag="tmp")
                                nc.vector.tensor_tensor(
                                    t2[:], rp[:], sin_c[:], MULT)
                                nc.vector.tensor_tensor(
                                    store[hl][:, tr], t1[:], t2[:], ADD)

                        for half in range(2):
                            pv = qk_ps.tile([128, 2 * W], F32,
                                            name=f"vp{b}{j}{half}", tag="qk")
                            for sub in range(2):
                                ts4 = half * 2 + sub
                                cs0 = sub * W
                                for kt in range(NKT):
                                    nc.tensor.matmul(
                                        pv[:, cs0:cs0 + W],
                                        xt[kt][:, ts4 * 128:(ts4 + 1) * 128],
                                        wv_sb[:, kt * W:(kt + 1) * W],
                                        start=(kt == 0), stop=(kt == NKT - 1),
                                        skip_group_check=True)
                            for sub in range(2):
                                ts4 = half * 2 + sub
                                tt = j * 4 + ts4
                                for hl in range(HPC):
                                    nc.scalar.copy(
                                        v_st[b][hl][:,
                                                    tt * 128:(tt + 1) * 128],
                                        pv[:, sub * W + hl * HD:
                                           sub * W + (hl + 1) * HD])

                    # ---------- pipeline ----------
                    # HAM warm-up: ~80 tiny matmuls on a zeroed scratch
                    # keep the PE active through the DMA-bound startup so
                    # the first real matmuls run at 2.4GHz, not 1.2
                    wsc = tmp_pool.tile([128, 64], BF16, name="warm_sc",
                                        tag="tmp")
                    nc.vector.memzero(wsc[:])
                    wps = qk_ps.tile([128, 64], F32, name="warm_ps",
                                     tag="qk")
                    for _ in range(80):
                        nc.tensor.matmul(wps[0:64, :], wsc[:, 0:64],
                                         wsc[:],
                                         start=True, stop=True,
                                         skip_group_check=True)
                    # unit order: (0,0,0) (0,0,1) (0,1,0) (0,0,2) (0,1,1)
                    # (0,0,3) (0,1,2)[B(0,0,3)] | (1,0,0) (0,1,3) (1,0,1)
                    # [B(0,1,3)] (1,1,0) (1,0,2) (1,1,1) (1,0,3) (1,1,2)
                    # [B(1,0,3)] | (1,1,3) [B(1,1,2)] + explicit B(1,1,3)
                    for j in range(NCH):            # batch-0 chunks
                        p1_chunk(0, j)
                        run_unit(0, 0, j)
                        if j == 0:
                            # dummy collective: absorbs the CC-engine
                            # priming cost so round 0 starts promptly
                            nc.sync.dma_start(warm_in[:], mask_sb[0:8, :])
                            nc.gpsimd.collective_compute(
                                "AllToAll",
                                mybir.AluOpType.bypass,
                                replica_groups=[list(range(NCORES))],
                                ins=[warm_in[:].opt()],
                                outs=[warm_out[:].opt()],
                            )
                        else:
                            run_unit(0, 1, j - 1)
                    fire_round(0, 0)                # after B(0,0,3)

                    for j in range(NCH):            # batch-1 chunks
                        p1_chunk(1, j)
                        run_unit(1, 0, j)
                        if j == 0:
                            run_unit(0, 1, 3)
                        else:
                            if j == 1:
                                fire_round(1, 0)    # after B(0,1,3)
                            if j == 3:
                                tc.no_sync_barrier()
                                a = gen_A(1, 1, 2)
                                bg = gen_B(pending[0])
                                drive(a, bg,
                                      quarter_pass_gen(0, 0, fcs=(0, 1)))
                                pending[0] = st_state["unit"]
                            else:
                                run_unit(1, 1, j - 1)
                        if j == 2:
                            oc_load(0, 0)
                        elif j == 3:
                            oc_load(1, 0)
                    fire_round(0, 1)                # after B(1,0,3)

                run_unit(1, 1, 3)
                drive(quarter_pass_gen(0, 0, fcs=(2, 3)),
                      gen_B(pending[0]))             # B(1,1,3) interleaved
                pending[0] = None
                fire_round(1, 1)

                for f in reversed(wfrees):
                    f()
                for f in reversed(deadfrees):
                    f()

                with tc.tile_pool(name="wop1", bufs=32) as wo1_pool:
                    # parity-1 weights stream in consumption order while
                    # quarter_pass(1, 0) starts chewing
                    for fc in range(D // TCH):
                        for s in range(NCORES):
                            wo_load(s, fc, 1, nc.gpsimd, wo1_pool)
                    tc.no_sync_barrier()
                    quarter_pass(1, 0)
                    oc_load(0, 1)
                    tc.no_sync_barrier()
                    quarter_pass(0, 1)
                    oc_load(1, 1)
                    tc.no_sync_barrier()
                    quarter_pass(1, 1)

                for f in reversed(keepfrees):
                    f()

        for f in reversed(frees):
            f()

    nc.compile()
    return nc


def _host_inputs(x, qkv_w, qkv_b, out_w, out_b):
    import ml_dtypes

    f32 = np.float32
    bf16 = ml_dtypes.bfloat16
    fp16 = np.float16

    x = np.asarray(x, dtype=f32)
    qkv_w = np.asarray(qkv_w, dtype=f32)
    qkv_b = np.asarray(qkv_b, dtype=f32)
    out_w = np.asarray(out_w, dtype=f32)
    out_b = np.asarray(out_b, dtype=f32)

    xT = np.ascontiguousarray(x.transpose(0, 2, 1)).astype(bf16)  # [B, D, T]
    qkv_wT = np.ascontiguousarray(qkv_w.T)                        # [D, 3D]
    wo_h = np.ascontiguousarray(out_w.T).astype(bf16)             # [D, D]
    # v-bias folds into the out bias exactly (softmax rows sum to 1)
    bv_full = qkv_b[2 * D:3 * D].astype(np.float64)
    bo_f = out_b.astype(np.float64) + out_w.astype(np.float64) @ bv_full
    bo_h = bo_f.astype(fp16).reshape(1, D)

    half = HD // 2
    freq = (1.0 / (10000.0 ** (np.arange(half, dtype=np.float64) / half)))
    ang = freq[:, None] * np.arange(T, dtype=np.float64)[None, :]
    cos_h = np.cos(ang)
    sin_h = np.sin(ang)
    cosT = np.concatenate([cos_h, cos_h], axis=0).astype(fp16)
    sinT = np.concatenate([sin_h, sin_h], axis=0).astype(fp16)

    P = np.zeros((HD, HD), dtype=f32)
    P[np.arange(half), np.arange(half) + half] = -1.0
    P[np.arange(half) + half, np.arange(half)] = 1.0
    pt_h = np.ascontiguousarray(P.T).astype(fp16)

    mask = np.where(np.arange(HD)[:, None] > np.arange(HD)[None, :],
                    f32(0.0), f32(1.0)).astype(bf16)
    onec_h = np.ones((HD, 1), dtype=fp16)
    oner_h = np.ones((1, HD), dtype=fp16)
    expb_h = np.full((HD, 1), EXPB, dtype=f32)

    in_maps = []
    for c in range(NCORES):
        g0 = c * W
        wq_c = np.ascontiguousarray(qkv_wT[:, g0:g0 + W]).astype(bf16)
        wk_c = np.ascontiguousarray(qkv_wT[:, D + g0:D + g0 + W]).astype(bf16)
        wv_c = np.ascontiguousarray(
            qkv_wT[:, 2 * D + g0:2 * D + g0 + W]).astype(bf16)
        bq_c = qkv_b[g0:g0 + W].reshape(HPC, HD, 1)
        bk_c = qkv_b[D + g0:D + g0 + W].reshape(HPC, HD, 1)
        bqk_c = np.ascontiguousarray(np.concatenate([bq_c, bk_c], axis=0))
        in_maps.append({
            "xT": xT, "wq": wq_c, "wk": wk_c, "wv": wv_c,
            "bqk": bqk_c,
            "wo": wo_h, "bo": bo_h, "cosT": cosT, "sinT": sinT,
            "pt": pt_h, "maskT": mask,
            "onec16": onec_h, "oner": oner_h, "expb": expb_h,
        })
    return in_maps


def kernel(x, qkv_w, qkv_b, out_w, out_b):
    from concourse.bass_utils import run_bass_kernel_spmd

    if "nc" not in _CACHE:
        _CACHE["nc"] = _build_module()
    nc = _CACHE["nc"]

    in_maps = _host_inputs(x, qkv_w, qkv_b, out_w, out_b)
    res = run_bass_kernel_spmd(nc, in_maps, core_ids=list(range(NCORES)))
    out = np.empty((B, T, D), dtype=np.float32)
    for p in range(NCORES):
        yp = np.asarray(res.results[p]["y"], dtype=np.float32)
        out[0, p * QT:(p + 1) * QT] = yp[0:QT]
        out[1, p * QT:(p + 1) * QT] = yp[QT:RPC]
    return out


# revision 35
# speedup vs baseline: 1.0301x; 1.0301x over previous
"""Multi-head causal self-attention (B=2, T=2048, D=2048, 16 heads, RoPE)
on 8 Trainium2 NeuronCores — v4.

Changes vs v3 (470us):
* Lag schedule: head-1 attention units trail head-0 by one chunk inside
  both batch loops, so every chunk iteration carries QKV + ~2 attention
  units and the post-loop tail is a single unit.
* Four quarter-size AllToAlls, one per (head-parity, batch), each over
  [8*128, 256]-token blocks.  They fire as soon as their 4 units finish
  (~150/190/288/300us) and pipeline on the CC engine, so the tail
  collective is 512KB instead of 1MB and starts far earlier.
* Out-projection runs as four quarter-passes (one per round): parity-0
  parks partial sums in fp16, parity-1 adds and stores.  Each core now
  owns one 256-token slice of BOTH batches (y rows = [b0 slice; b1
  slice]); the host gather interleaves accordingly.
* tc.no_sync_barrier() before each quarter-pass emission: the Tile
  scheduler under-models collective latency and otherwise hoists
  pass matmuls into the attention PE stream, where their LDWEIGHTS
  stall the strict-FIFO PE queue (v3 lost 28us to this).
"""

import numpy as np

B = 2
T = 2048
D = 2048
H = 16
HD = 128
NCORES = 8
HPC = H // NCORES   # heads per core (2)
NKT = D // 128      # contraction tiles (16)
TCH = 512           # t-chunk width
NCH = T // TCH      # chunks per batch (4)
W = HPC * HD        # per-core qkv feature width (256)
QT = T // NCORES    # per-core token slice for out-proj (256)
RPC = B * QT        # output rows per core (512)
SCALE = 1.0 / np.sqrt(HD)
EXPB = -float(np.log(16.0))   # exp pre-bias keeps the fp16 denom small

_CACHE = {}


def _build_module():
    import concourse.bacc as bacc
    import concourse.mybir as mybir
    import concourse.tile as tile

    F32 = mybir.dt.float32
    BF16 = mybir.dt.bfloat16
    FP16 = mybir.dt.float16
    ADD = mybir.AluOpType.add
    MULT = mybir.AluOpType.mult
    AF = mybir.ActivationFunctionType

    nc = bacc.Bacc("TRN2", target_bir_lowering=False, debug=False,
                   num_devices=NCORES)

    # ---- I/O ----
    xT = nc.dram_tensor("xT", [B, D, T], BF16, kind="ExternalInput")
    wq = nc.dram_tensor("wq", [D, W], BF16, kind="ExternalInput")
    wk = nc.dram_tensor("wk", [D, W], BF16, kind="ExternalInput")
    wv = nc.dram_tensor("wv", [D, W], BF16, kind="ExternalInput")
    bqk = nc.dram_tensor("bqk", [2 * HPC, HD, 1], F32, kind="ExternalInput")
    wo = nc.dram_tensor("wo", [D, D], BF16, kind="ExternalInput")
    bo = nc.dram_tensor("bo", [1, D], FP16, kind="ExternalInput")
    cosT = nc.dram_tensor("cosT", [HD, T], FP16, kind="ExternalInput")
    sinT = nc.dram_tensor("sinT", [HD, T], FP16, kind="ExternalInput")
    pt = nc.dram_tensor("pt", [HD, HD], FP16, kind="ExternalInput")
    maskT = nc.dram_tensor("maskT", [HD, HD], BF16, kind="ExternalInput")
    onec16 = nc.dram_tensor("onec16", [HD, 1], FP16, kind="ExternalInput")
    oner = nc.dram_tensor("oner", [1, HD], FP16, kind="ExternalInput")
    expb = nc.dram_tensor("expb", [HD, 1], F32, kind="ExternalInput")
    y = nc.dram_tensor("y", [RPC, D], FP16, kind="ExternalOutput")

    with tile.TileContext(nc) as tc:
        frees = []

        def single(shape, dtype, name, flist=frees):
            t, free = tc.tile(shape, dtype, name=name)
            flist.append(free)
            return t

        pt_sb = single([HD, HD], FP16, "pt_sb")
        mask_sb = single([HD, HD], BF16, "mask_sb")
        onec_sb = single([HD, 1], FP16, "onec_sb")
        oner_sb = single([1, HD], FP16, "oner_sb")
        bqk_sb = single([HD, 2 * HPC], F32, "bqk_sb")
        expb_sb = single([HD, 1], F32, "expb_sb")
        bo_sb = single([1, D], FP16, "bo_sb")

        with tc.tile_pool(name="dram", bufs=1, space="DRAM") as dram:
            # round (hl, b): block p = head(2s+hl) of batch b, token slice
            # [p*QT:(p+1)*QT]
            bounce_in = [[dram.tile([NCORES * HD, QT], BF16,
                                    name=f"bi{hl}{b}") for b in range(B)]
                         for hl in range(HPC)]
            bounce_out = [[dram.tile([NCORES * HD, QT], BF16,
                                     name=f"bu{hl}{b}") for b in range(B)]
                          for hl in range(HPC)]
            warm_in = dram.tile([NCORES, 128], BF16, name="warm_in")
            warm_out = dram.tile([NCORES, 128], BF16, name="warm_out")

            with tc.tile_pool(name="qk_ps", bufs=3, space="PSUM") as qk_ps, \
                 tc.tile_pool(name="rot_ps", bufs=2, space="PSUM") as rot_ps, \
                 tc.tile_pool(name="st_ps", bufs=2, space="PSUM") as st_ps, \
                 tc.tile_pool(name="ot_ps", bufs=1, space="PSUM") as ot_ps, \
                 tc.tile_pool(name="et", bufs=24) as et_pool, \
                 tc.tile_pool(name="ets", bufs=2) as ets_pool, \
                 tc.tile_pool(name="nrm", bufs=4) as nrm_pool, \
                 tc.tile_pool(name="oto", bufs=2) as oto_pool, \
                 tc.tile_pool(name="wop", bufs=32) as wo_pool, \
                 tc.tile_pool(name="ocp", bufs=16) as oc_pool, \
                 tc.tile_pool(name="osp", bufs=16) as osp_pool, \
                 tc.tile_pool(name="ozp", bufs=2) as oz_pool:

                # stack-ordered singles: tiles alive into the post-loop
                # tail at the bottom, so the rest frees before wo1 opens.
                v_st = [[None] * HPC for _ in range(B)]
                q_st = [[None] * HPC for _ in range(B)]
                k_st = [[None] * HPC for _ in range(B)]
                keepfrees = []
                v_st[1][1] = single([128, T], BF16, "v11", keepfrees)
                q_st[1][1] = single([128, T], FP16, "q11", keepfrees)
                k_st[1][1] = single([128, T], FP16, "k11", keepfrees)
                deadfrees = []
                v_st[0][0] = single([128, T], BF16, "v00", deadfrees)
                v_st[0][1] = single([128, T], BF16, "v01", deadfrees)
                v_st[1][0] = single([128, T], BF16, "v10", deadfrees)
                q_st[0][0] = single([128, T], FP16, "q00", deadfrees)
                q_st[0][1] = single([128, T], FP16, "q01", deadfrees)
                q_st[1][0] = single([128, T], FP16, "q10", deadfrees)
                k_st[0][0] = single([128, T], FP16, "k00", deadfrees)
                k_st[0][1] = single([128, T], FP16, "k01", deadfrees)
                k_st[1][0] = single([128, T], FP16, "k10", deadfrees)
                wfrees = []
                wq_sb = single([128, NKT * W], BF16, "wq_sb", wfrees)
                wk_sb = single([128, NKT * W], BF16, "wk_sb", wfrees)
                wv_sb = single([128, NKT * W], BF16, "wv_sb", wfrees)

                # out-proj weight tiles: wts[(s, fc, hl)] = wo rows of head
                # 2s+hl, out-cols [fc*TCH:(fc+1)*TCH]
                wts = {}

                def wo_load(s, fc, hl, eng, pool):
                    kt = HPC * s + hl
                    t_ = pool.tile([128, TCH], BF16,
                                   name=f"wo{kt}_{fc}", tag="wo")
                    eng.dma_start(
                        t_[:],
                        wo.ap()[kt * 128:(kt + 1) * 128,
                                fc * TCH:(fc + 1) * TCH])
                    wts[s, fc, hl] = t_

                # received attention outputs: oc[(hl, b)][s]
                oc = {}

                def oc_load(hl, b):
                    tl = []
                    for s in range(NCORES):
                        t_ = oc_pool.tile([128, QT], BF16,
                                          name=f"oc{hl}{b}_{s}", tag="oc")
                        nc.gpsimd.dma_start(
                            t_[:],
                            bounce_out[hl][b][s * HD:(s + 1) * HD, :])
                        tl.append(t_)
                    oc[hl, b] = tl

                # ---------- emission helpers ----------
                st_state = {}

                def gen_A(b, hl, c):
                    """Scores + exp + fp16 running denominator (one unit).
                    Yields once per k-tile; tail emits den/recip/rcr."""
                    nk = 4 * c + 4
                    q0 = c * TCH
                    ets = ets_pool.tile([128, TCH], FP16,
                                        name=f"ets{b}{hl}{c}", tag="ets")
                    et_list = []
                    for k in range(nk):
                        off = max(0, (k - 4 * c) * 128)
                        st = st_ps.tile([128, TCH], F32,
                                        name=f"st{b}{hl}{c}{k}", tag="st")
                        nc.tensor.matmul(
                            st[:, off:TCH],
                            k_st[b][hl][:, k * 128:(k + 1) * 128],
                            q_st[b][hl][:, q0 + off:q0 + TCH],
                            start=True, stop=True,
                            skip_group_check=True)
                        et = et_pool.tile([128, TCH], BF16,
                                          name=f"et{b}{hl}{c}{k}", tag="et")
                        nc.scalar.activation(
                            et[:, off:TCH], st[:, off:TCH],
                            AF.Exp, bias=expb_sb[:, 0:1],
                            scale=float(SCALE))
                        if k >= 4 * c:
                            nc.vector.tensor_tensor(
                                et[:, off:off + 128],
                                et[:, off:off + 128], mask_sb[:], MULT)
                        if k == 0:
                            nc.vector.tensor_copy(ets[:], et[:])
                        else:
                            nc.vector.tensor_tensor(
                                ets[:, off:TCH], ets[:, off:TCH],
                                et[:, off:TCH], ADD)
                        et_list.append((et, off))
                        yield
                    den = rot_ps.tile([1, TCH], F32,
                                      name=f"den{b}{hl}{c}", tag="rot")
                    nc.tensor.matmul(den[0:1, :], onec_sb[:], ets[:],
                                     start=True, stop=True,
                                     skip_group_check=True)
                    rc = nrm_pool.tile([1, TCH], F32,
                                       name=f"rc{b}{hl}{c}", tag="rc")
                    rscr = nrm_pool.tile([1, TCH], F32,
                                         name=f"rs{b}{hl}{c}", tag="rc")
                    nc.vector.reciprocal_approx_accurate(
                        rc[:], den[0:1, :], rscr[:])
                    rcr = nrm_pool.tile([1, TCH], FP16,
                                        name=f"rr{b}{hl}{c}", tag="rcr")
                    nc.scalar.copy(rcr[:], rc[:])
                    st_state["unit"] = (b, hl, c, et_list, rcr)

                def gen_B(unit):
                    """AV accumulation + normalization + A2A scatter."""
                    b, hl, c, et_list, rcr = unit
                    nk = len(et_list)
                    bc = rot_ps.tile([128, TCH], F32,
                                     name=f"bc{b}{hl}{c}", tag="rot")
                    nc.tensor.matmul(bc[:], oner_sb[:], rcr[:],
                                     start=True, stop=True,
                                     skip_group_check=True)
                    bcs = nrm_pool.tile([128, TCH], F32,
                                        name=f"bs{b}{hl}{c}", tag="bcs")
                    nc.scalar.copy(bcs[:], bc[:])
                    ot = ot_ps.tile([128, TCH], F32,
                                    name=f"ot{b}{hl}{c}", tag="ot")
                    for k in range(nk):
                        et, off = et_list[k]
                        nc.tensor.matmul(
                            ot[:, off:TCH],
                            v_st[b][hl][:, k * 128:(k + 1) * 128],
                            et[:, off:TCH],
                            start=(k == 0), stop=(k == nk - 1),
                            skip_group_check=True)
                        yield
                    otn = oto_pool.tile([128, TCH], BF16,
                                        name=f"on{b}{hl}{c}", tag="otn")
                    nc.vector.tensor_tensor(otn[:], ot[:], bcs[:], MULT)
                    # chunk c covers token blocks 2c, 2c+1 of this batch
                    nc.sync.dma_start(
                        bounce_in[hl][b][2 * c * HD:(2 * c + 1) * HD, :],
                        otn[:, 0:QT])
                    nc.sync.dma_start(
                        bounce_in[hl][b][(2 * c + 1) * HD:(2 * c + 2) * HD, :],
                        otn[:, QT:TCH])

                def drive(a_gen, b_gen, c_gen=None):
                    """Alternate B, A (and optional filler) steps — B
                    first: its deps are older, so the PE queue head never
                    blocks long."""
                    gens = [g for g in (b_gen, a_gen, c_gen)
                            if g is not None]
                    while gens:
                        for g in list(gens):
                            try:
                                next(g)
                            except StopIteration:
                                gens.remove(g)

                def fire_round(hl, b):
                    nc.gpsimd.collective_compute(
                        "AllToAll",
                        mybir.AluOpType.bypass,
                        replica_groups=[list(range(NCORES))],
                        ins=[bounce_in[hl][b][:].opt()],
                        outs=[bounce_out[hl][b][:].opt()],
                    )

                pending = [None]

                def run_unit(b, hl, c):
                    a = gen_A(b, hl, c)
                    bgen = (gen_B(pending[0])
                            if pending[0] is not None else None)
                    drive(a, bgen)
                    pending[0] = st_state["unit"]

                def quarter_pass_gen(hl, b, fcs=None):
                    first = (hl == 0)
                    for fc in (fcs if fcs is not None else range(D // TCH)):
                        for rt in range(QT // 128):
                            po = qk_ps.tile([128, TCH], F32,
                                            name=f"po{hl}{b}{fc}{rt}",
                                            tag="qk")
                            if first:
                                nc.tensor.matmul(
                                    po[:], oner_sb[:],
                                    bo_sb[0:1, fc * TCH:(fc + 1) * TCH],
                                    start=True, stop=False,
                                    skip_group_check=True)
                            for s in range(NCORES):
                                nc.tensor.matmul(
                                    po[:],
                                    oc[hl, b][s][:, rt * 128:(rt + 1) * 128],
                                    wts[s, fc, hl][:],
                                    start=(not first and s == 0),
                                    stop=(s == NCORES - 1),
                                    skip_group_check=True)
                            if first:
                                osp_t = osp_pool.tile(
                                    [128, TCH], FP16,
                                    name=f"os{b}{fc}{rt}", tag="osp")
                                if (fc + rt) % 2:
                                    nc.scalar.copy(osp_t[:], po[:])
                                else:
                                    nc.vector.tensor_copy(osp_t[:], po[:])
                                osp[b, fc, rt] = osp_t
                            else:
                                oz = oz_pool.tile(
                                    [128, TCH], FP16,
                                    name=f"oz{b}{fc}{rt}", tag="oz")
                                nc.vector.tensor_tensor(
                                    oz[:], po[:], osp[b, fc, rt][:], ADD)
                                r0 = b * QT + rt * 128
                                nc.sync.dma_start(
                                    y.ap()[r0:r0 + 128,
                                           fc * TCH:(fc + 1) * TCH],
                                    oz[:])
                            yield

                def quarter_pass(hl, b, fcs=None):
                    for _ in quarter_pass_gen(hl, b, fcs):
                        pass

                osp = {}

                with tc.tile_pool(name="xt", bufs=17) as xt_pool, \
                     tc.tile_pool(name="cs", bufs=4) as cs_pool, \
                     tc.tile_pool(name="tmp", bufs=4) as tmp_pool:

                    cs_cache = {}

                    def p1_chunk(b, j):
                        """QKV projection + RoPE for chunk j of batch b."""
                        tr = slice(j * TCH, (j + 1) * TCH)
                        if b == 0 and j == 0:
                            nc.sync.dma_start(pt_sb[:], pt.ap()[:, :])
                            for i in range(2 * HPC):
                                nc.gpsimd.dma_start(bqk_sb[:, i:i + 1],
                                                    bqk.ap()[i])
                            nc.gpsimd.dma_start(oner_sb[:], oner.ap()[:, :])
                        xt = []
                        for kt in range(NKT):
                            xtile = xt_pool.tile([128, TCH], BF16,
                                                 name=f"xt{b}{j}_{kt}",
                                                 tag="xt")
                            xeng = (nc.gpsimd if (b == 0 and j == 0
                                                  and kt >= 8) else nc.sync)
                            xeng.dma_start(
                                xtile[:],
                                xT.ap()[b, kt * 128:(kt + 1) * 128, tr])
                            xt.append(xtile)
                            if b == 0 and j == 0 and kt == 0:
                                # issue order follows the consumption
                                # order of the first chains
                                def wload(wsb, wdr, k0, k1):
                                    if k1 - k0 == 1:
                                        nc.scalar.dma_start(
                                            wsb[:, k0 * W:k1 * W],
                                            wdr.ap()[k0 * 128:k1 * 128, :])
                                        return
                                    dst = wsb[:, k0 * W:k1 * W]
                                    dst = dst.rearrange(
                                        "p (k w) -> p k w", k=k1 - k0)
                                    srcw = wdr.ap()[k0 * 128:k1 * 128, :]
                                    srcw = srcw.rearrange(
                                        "(k p) w -> p k w", p=128)
                                    nc.scalar.dma_start(dst, srcw)
                                for wsb, wdr, k0, k1 in (
                                        (wq_sb, wq, 0, 1),
                                        (wq_sb, wq, 1, 6),
                                        (wk_sb, wk, 0, 1),
                                        (wq_sb, wq, 6, 11),
                                        (wq_sb, wq, 11, 16),
                                        (wk_sb, wk, 1, 6),
                                        (wk_sb, wk, 6, 11),
                                        (wk_sb, wk, 11, 16),
                                        (wv_sb, wv, 0, 6),
                                        (wv_sb, wv, 6, 11),
                                        (wv_sb, wv, 11, 16)):
                                    wload(wsb, wdr, k0, k1)
                        g = 4 * b + j
                        loads = ([0, 1] if g == 0 else
                                 [g + 1] if g + 1 < 2 * NCH else [])
                        for gl in loads:
                            jl = gl % NCH
                            trl = slice(jl * TCH, (jl + 1) * TCH)
                            cos_l = cs_pool.tile([HD, TCH], FP16,
                                                 name=f"cos{gl}", tag="cs")
                            nc.gpsimd.dma_start(cos_l[:], cosT.ap()[:, trl])
                            sin_l = cs_pool.tile([HD, TCH], FP16,
                                                 name=f"sin{gl}", tag="cs")
                            nc.gpsimd.dma_start(sin_l[:], sinT.ap()[:, trl])
                            cs_cache[gl] = (cos_l, sin_l)
                        cos_c, sin_c = cs_cache.pop(g)
                        if b == 0 and j == 0:
                            nc.gpsimd.dma_start(mask_sb[:], maskT.ap()[:, :])
                            nc.gpsimd.dma_start(onec_sb[:], onec16.ap()[:, :])
                            nc.gpsimd.dma_start(expb_sb[:], expb.ap()[:, :])
                            nc.scalar.dma_start(bo_sb[:], bo.ap()[:, :])
                        elif 1 <= g <= 5:
                            # prefetch parity-0 out-proj weights, spread
                            # thin so collectives never compete with a
                            # concentrated DMA burst
                            todo = [(s, fc) for fc in range(D // TCH)
                                    for s in range(NCORES)][(g - 1) * 7:
                                                            g * 7]
                            for s, fc in todo:
                                wo_load(s, fc, 0, nc.gpsimd, wo_pool)

                        for which, w_sb, store in (
                                ("q", wq_sb, q_st[b]), ("k", wk_sb, k_st[b])):
                            for hl in range(HPC):
                                ps = qk_ps.tile([128, TCH], F32,
                                                name=f"{which}p{b}{j}{hl}",
                                                tag="qk")
                                for kt in range(NKT):
                                    col = kt * W + hl * HD
                                    nc.tensor.matmul(
                                        ps[:], w_sb[:, col:col + HD],
                                        xt[kt][:],
                                        start=(kt == 0),
                                        stop=(kt == NKT - 1),
                                        skip_group_check=True)
                                bcol = hl if which == "q" else HPC + hl
                                qtmp = tmp_pool.tile(
                                    [128, TCH], FP16,
                                    name=f"{which}t{b}{j}{hl}", tag="tmp")
                                nc.scalar.activation(
                                    qtmp[:], ps[:], AF.Identity,
                                    bias=bqk_sb[:, bcol:bcol + 1], scale=1.0)
                                rp = rot_ps.tile([128, TCH], F32,
                                                 name=f"rp{b}{j}{hl}{which}",
                                                 tag="rot")
                                nc.tensor.matmul(rp[:], pt_sb[:], qtmp[:],
                                                 start=True, stop=True,
                                                 skip_group_check=True)
                                t1 = tmp_pool.tile([128, TCH], FP16,
                                                   name=f"t1_{b}{j}{hl}",
                                                   tag="tmp")
                                nc.vector.tensor_tensor(
                                    t1[:], qtmp[:], cos_c[:], MULT)
                                t2 = tmp_pool.tile([128, TCH], FP16,
                                                   name=f"t2_{b}{j}{hl}",
                                                   tag="tmp")
                                nc.vector.tensor_tensor(
                                    t2[:], rp[:], sin_c[:], MULT)
                                nc.vector.tensor_tensor(
                                    store[hl][:, tr], t1[:], t2[:], ADD)

                        for half in range(2):
                            pv = qk_ps.tile([128, 2 * W], F32,
                                            name=f"vp{b}{j}{half}", tag="qk")
                            for sub in range(2):
                                ts4 = half * 2 + sub
                                cs0 = sub * W
                                for kt in range(NKT):
                                    nc.tensor.matmul(
                                        pv[:, cs0:cs0 + W],
                                        xt[kt][:, ts4 * 128:(ts4 + 1) * 128],
                                        wv_sb[:, kt * W:(kt + 1) * W],
                                        start=(kt == 0), stop=(kt == NKT - 1),
                                        skip_group_check=True)
                            for sub in range(2):
                                ts4 = half * 2 + sub
                                tt = j * 4 + ts4
                                for hl in range(HPC):
                                    nc.scalar.copy(
                                        v_st[b][hl][:,
                                                    tt * 128:(tt + 1) * 128],
                                        pv[:, sub * W + hl * HD:
                                           sub * W + (hl + 1) * HD])

                    # ---------- pipeline ----------
                    # HAM warm-up: ~80 tiny matmuls on a zeroed scratch
                    # keep the PE active through the DMA-bound startup so
                    # the first real matmuls run at 2.4GHz, not 1.2
                    wsc = tmp_pool.tile([128, 64], BF16, name="warm_sc",
                                        tag="tmp")
                    nc.vector.memzero(wsc[:])
                    wps = qk_ps.tile([128, 64], F32, name="warm_ps",
                                     tag="qk")
                    for _ in range(80):
                        nc.tensor.matmul(wps[0:64, :], wsc[:, 0:64],
                                         wsc[:],
                                         start=True, stop=True,
                                         skip_group_check=True)
                    # unit order: (0,0,0) (0,0,1) (0,1,0) (0,0,2) (0,1,1)
                    # (0,0,3) (0,1,2)[B(0,0,3)] | (1,0,0) (0,1,3) (1,0,1)
                    # [B(0,1,3)] (1,1,0) (1,0,2) (1,1,1) (1,0,3) (1,1,2)
                    # [B(1,0,3)] | (1,1,3) [B(1,1,2)] + explicit B(1,1,3)
                    for j in range(NCH):            # batch-0 chunks
                        p1_chunk(0, j)
                        run_unit(0, 0, j)
                        if j == 0:
                            # dummy collective: absorbs the CC-engine
                            # priming cost so round 0 starts promptly
                            nc.sync.dma_start(warm_in[:], mask_sb[0:8, :])
                            nc.gpsimd.collective_compute(
                                "AllToAll",
                                mybir.AluOpType.bypass,
                                replica_groups=[list(range(NCORES))],
                                ins=[warm_in[:].opt()],
                                outs=[warm_out[:].opt()],
                            )
                        else:
                            run_unit(0, 1, j - 1)
                    fire_round(0, 0)                # after B(0,0,3)

                    for j in range(NCH):            # batch-1 chunks
                        p1_chunk(1, j)
                        run_unit(1, 0, j)
                        if j == 0:
                            run_unit(0, 1, 3)
                        else:
                            if j == 1:
                                fire_round(1, 0)    # after B(0,1,3)
                            run_unit(1, 1, j - 1)
                        if j == 2:
                            oc_load(0, 0)
                        elif j == 3:
                            oc_load(1, 0)
                    fire_round(0, 1)                # after B(1,0,3)

                tc.no_sync_barrier()
                quarter_pass(0, 0, fcs=(0, 1))
                run_unit(1, 1, 3)
                drive(quarter_pass_gen(0, 0, fcs=(2, 3)),
                      gen_B(pending[0]))             # B(1,1,3) interleaved
                pending[0] = None
                fire_round(1, 1)

                for f in reversed(wfrees):
                    f()
                for f in reversed(deadfrees):
                    f()

                with tc.tile_pool(name="wop1", bufs=32) as wo1_pool:
                    # parity-1 weights stream in consumption order while
                    # quarter_pass(1, 0) starts chewing
                    for fc in range(D // TCH):
                        for s in range(NCORES):
                            wo_load(s, fc, 1, nc.gpsimd, wo1_pool)
                    tc.no_sync_barrier()
                    quarter_pass(1, 0)
                    oc_load(0, 1)
                    tc.no_sync_barrier()
                    quarter_pass(0, 1)
                    oc_load(1, 1)
                    tc.no_sync_barrier()
                    quarter_pass(1, 1)

                for f in reversed(keepfrees):
                    f()

        for f in reversed(frees):
            f()

    nc.compile()
    return nc


def _host_inputs(x, qkv_w, qkv_b, out_w, out_b):
    import ml_dtypes

    f32 = np.float32
    bf16 = ml_dtypes.bfloat16
    fp16 = np.float16

    x = np.asarray(x, dtype=f32)
    qkv_w = np.asarray(qkv_w, dtype=f32)
    qkv_b = np.asarray(qkv_b, dtype=f32)
    out_w = np.asarray(out_w, dtype=f32)
    out_b = np.asarray(out_b, dtype=f32)

    xT = np.ascontiguousarray(x.transpose(0, 2, 1)).astype(bf16)  # [B, D, T]
    qkv_wT = np.ascontiguousarray(qkv_w.T)                        # [D, 3D]
    wo_h = np.ascontiguousarray(out_w.T).astype(bf16)             # [D, D]
    # v-bias folds into the out bias exactly (softmax rows sum to 1)
    bv_full = qkv_b[2 * D:3 * D].astype(np.float64)
    bo_f = out_b.astype(np.float64) + out_w.astype(np.float64) @ bv_full
    bo_h = bo_f.astype(fp16).reshape(1, D)

    half = HD // 2
    freq = (1.0 / (10000.0 ** (np.arange(half, dtype=np.float64) / half)))
    ang = freq[:, None] * np.arange(T, dtype=np.float64)[None, :]
    cos_h = np.cos(ang)
    sin_h = np.sin(ang)
    cosT = np.concatenate([cos_h, cos_h], axis=0).astype(fp16)
    sinT = np.concatenate([sin_h, sin_h], axis=0).astype(fp16)

    P = np.zeros((HD, HD), dtype=f32)
    P[np.arange(half), np.arange(half) + half] = -1.0
    P[np.arange(half) + half, np.arange(half)] = 1.0
    pt_h = np.ascontiguousarray(P.T).astype(fp16)

    mask = np.where(np.arange(HD)[:, None] > np.arange(HD)[None, :],
                    f32(0.0), f32(1.0)).astype(bf16)
    onec_h = np.ones((HD, 1), dtype=fp16)
    oner_h = np.ones((1, HD), dtype=fp16)
    expb_h = np.full((HD, 1), EXPB, dtype=f32)

    in_maps = []
    for c in range(NCORES):
        g0 = c * W
        wq_c = np.ascontiguousarray(qkv_wT[:, g0:g0 + W]).astype(bf16)
        wk_c = np.ascontiguousarray(qkv_wT[:, D + g0:D + g0 + W]).astype(bf16)
        wv_c = np.ascontiguousarray(
            qkv_wT[:, 2 * D + g0:2 * D + g0 + W]).astype(bf16)
        bq_c = qkv_b[g0:g0 + W].reshape(HPC, HD, 1)
        bk_c = qkv_b[D + g0:D + g0 + W].reshape(HPC, HD, 1)
        bqk_c = np.ascontiguousarray(np.concatenate([bq_c, bk_c], axis=0))
        in_maps.append({
            "xT": xT, "wq": wq_c, "wk": wk_c, "wv": wv_c,
            "bqk": bqk_c,
            "wo": wo_h, "bo": bo_h, "cosT": cosT, "sinT": sinT,
            "pt": pt_h, "maskT": mask,
            "onec16": onec_h, "oner": oner_h, "expb": expb_h,
        })
    return in_maps


def kernel(x, qkv_w, qkv_b, out_w, out_b):
    from concourse.bass_utils import run_bass_kernel_spmd

    if "nc" not in _CACHE:
        _CACHE["nc"] = _build_module()
    nc = _CACHE["nc"]

    in_maps = _host_inputs(x, qkv_w, qkv_b, out_w, out_b)
    res = run_bass_kernel_spmd(nc, in_maps, core_ids=list(range(NCORES)))
    out = np.empty((B, T, D), dtype=np.float32)
    for p in range(NCORES):
        yp = np.asarray(res.results[p]["y"], dtype=np.float32)
        out[0, p * QT:(p + 1) * QT] = yp[0:QT]
        out[1, p * QT:(p + 1) * QT] = yp[QT:RPC]
    return out
